# revision 6
# baseline (speedup 1.0000x reference)
"""Trainium2 Bass kernel for nn_LookupLanguageModel (trigram backoff LM lookup).

v4: two gather rounds, no chained trigram round. HW indirect DMA consumes one
offset per partition, so all merging is done in the HOST TABLE LAYOUT
(query-independent repack of the read-only trie, per the replicate-the-tables
sharding hint):

  TU1[u] (608 i32), keyed by h1:
    [0:32)    ids of u's 32 children (invalid slots baked to -1: never match)
    [32:64)   children's backoff logs (f32 bits; for bw2 select)
    [64:352)  field-major POS block [9 fields x 32 candidates]: for candidate
              c, the 8 trigram-child ids of bigram node (h1->c) (f=0..7) and
              its num_children (f=8)
    [352:608) field-major NEG block [8 x 32]: the 8 trigram-child log bits
  TU2[u] (80 i32), keyed by (h2, slot): 8 blocks of [4 bigram-candidate ids
    (invalid baked to 2^17: out of scatter bounds), their 4 log bits, num2,
    bw1 bits]

After G_A=TU1[h1] and G_B=TU2[h2] land, the matched candidate cm (ids==h2) is
selected by masked mult + reduce along the candidate axis: max for the POS
fields (ids >= 0), min for NEG/bw2 (log bit patterns of negative floats are
negative ints -- exact, no float accumulation). That yields the entire
trigram record without another DMA round. Corrections go out as five
128-offset scatters (4 bigram + 1 trigram per partition), invalid/collided
slots pushed out of range and dropped via bounds_check. The dense baseline
write (logs[v]+bconst, from a host-replicated logs[0:V]) issues as vector op 6
so its completion no longer gates the scatters.

Layout: 128 partitions = 16 rows x 8 slots; partition p: row b=p>>3, slot
s=p&7 (output chunk [1024s,1024s+1024), trigram candidate s, bigram
candidates 4s..4s+3). Core c handles batch rows 16c..16c+15.
"""

import numpy as np

import concourse.bass as bass
import concourse.mybir as mybir
from concourse.bass import IndirectOffsetOnAxis
from concourse.bass_utils import run_bass_kernel_spmd

# ---- problem constants (must match the reference trie shapes) ----
V = 8192
N = 3
U = V + 1                   # 8193 unigram nodes
C2, C3 = 32, 8
B2 = U * C2                 # 262176 bigram nodes
B3 = B2 * C3                # 2097408 trigram nodes
XP = U + B2 + 1             # pointers length 270370
KI = B2 + B3                # ids length 2359584
NNODES = U + B2 + B3        # 2367777 (start of backoff weights inside logs)
LL = 2 * XP + (B3 - 1)      # logs length 2638147
BATCH = 128
NCORES = 8
BPC = BATCH // NCORES       # 16 rows per core

W1 = 608                    # TU1 record width
W2 = 80                     # TU2 record width

BIG = 1 << 18               # offset mask-out constant (> BPC*V - 1)
BADID = 1 << 17             # baked id for invalid bigram candidates
BOUNDS = BPC * V - 1        # max valid flat output element index per core

i32 = mybir.dt.int32
f32 = mybir.dt.float32

AX = mybir.AxisListType
OP = mybir.AluOpType

# hconst column map (int32 [128, HC])
HC_IDXA = 0      # h1 * W1
HC_IDXB = 1      # h2 * W2 + 10*s
HC_H2 = 2
HC_MS8 = 3       # 8 cols: (k==s)
HC_OFFB = 11     # b<<13
HC_OFFB2C = 12   # (b<<13) + 2*BIG
HC_SCOL = 13
HC = 14

# TU1 record layout
A_C1 = 0
A_BW2 = 32
A_POS = 64       # 9 fields x 32 candidates (tri ids f=0..7, num3 f=8)
A_NEG = 352      # 8 fields x 32 candidates (tri log bits)

# TU2 slot-block layout (10 per slot)
B_BI = 0
B_BL = 4
B_NUM2 = 8
B_BW1 = 9


def build_kernel() -> bass.Bass:
    nc = bass.Bass()

    hconst = nc.declare_dram_parameter("hconst", [128, HC], i32, isOutput=False)
    lurep = nc.declare_dram_parameter("lurep", [128, 1024], f32, isOutput=False)
    tu1 = nc.declare_dram_parameter("tu1", [U * W1, 1], i32, isOutput=False)
    tu2 = nc.declare_dram_parameter("tu2", [U * W2, 1], i32, isOutput=False)
    outp = nc.declare_dram_parameter("out", [BPC * V, 1], f32, isOutput=True)

    from contextlib import ExitStack

    with ExitStack() as ctx:
        _n = [0]

        def sb(shape, dt):
            _n[0] += 1
            return ctx.enter_context(nc.sbuf_tensor(f"t{_n[0]}", shape, dt))

        H = sb([128, HC], i32)
        LU = sb([128, 1024], f32)
        OUTT = sb([128, 1024], f32)
        GA = sb([128, W1], i32)
        GB = sb([128, 10], i32)
        OFF = sb([128, 5], i32)
        VAL = sb([128, 5], f32)

        MS8F = sb([128, 8], f32)
        EQ1 = sb([128, 32], i32)
        SCB = sb([128, 32], i32)
        SCP = sb([128, 288], i32)
        SCN = sb([128, 256], i32)
        TBP = sb([128, 9], i32)
        TBN = sb([128, 8], i32)
        SC4 = sb([128, 8], i32)
        SC5 = sb([128, 8], f32)
        EX = sb([128, 1], i32)
        BW2I = sb([128, 1], i32)
        BCONST = sb([128, 1], f32)
        TSID = sb([128, 1], i32)
        OFFT = sb([128, 1], i32)
        LTT = sb([128, 1], i32)
        LTTEX = sb([128, 1], i32)
        MT2 = sb([128, 1], i32)
        OFFBIB = sb([128, 4], i32)
        EQALL = sb([128, 32], i32)
        COL = sb([128, 4], i32)
        COLE = sb([128, 4], i32)

        sem = lambda name: ctx.enter_context(nc.semaphore(name))
        sv = sem("sv")
        sg = sem("sg")
        sem_h = sem("sem_h")
        sem_lu = sem("sem_lu")
        sem_ga = sem("sem_ga")
        sem_gb = sem("sem_gb")
        sem_out = sem("sem_out")
        sem_sc = sem("sem_sc")

        ctx.enter_context(nc.Block())

        g = nc.gpsimd
        v = nc.vector
        sy = nc.sync

        vcnt = [0]

        def vo(inst):
            if vcnt[0] > 0:
                inst.wait_op(sv, vcnt[0], "sem-ge")
            inst.then_inc(sv, 1)
            vcnt[0] += 1
            return inst

        def vw(*waits):
            for s_, val_ in waits:
                v.wait_ge(s_, val_)

        gcnt = [0]

        def go(inst):
            if gcnt[0] > 0:
                inst.wait_op(sg, gcnt[0], "sem-ge")
            inst.then_inc(sg, 1)
            gcnt[0] += 1
            return inst

        M_OUTT = 6
        M_OFFB = 17
        M_TSID = 19
        M_MT2 = 24

        # ================= sync: input DMAs + baseline write =================
        sy.dma_start(out=H[:, :], in_=hconst[:, :]).then_inc(sem_h, 16)
        sy.dma_start(out=LU[:, :], in_=lurep[:, :]).then_inc(sem_lu, 16)

        sy.wait_ge(sv, M_OUTT)
        sy.dma_start(
            out=outp[:, :].rearrange("(p f) o -> p (f o)", p=128),
            in_=OUTT[:, :],
        ).then_inc(sem_out, 16)

        # ================= gpsimd: 2 gathers + tail + 5 scatters =============
        def gather(dst, src, idx_ap, semh, *waits):
            for s_, val_ in waits:
                g.wait_ge(s_, val_)
            inst = g.indirect_dma_start(
                out=dst, out_offset=None,
                in_=src[:, :], in_offset=IndirectOffsetOnAxis(ap=idx_ap, axis=0),
            )
            inst.then_inc(semh, 16)
            return inst

        gather(GA[:, :], tu1, H[:, HC_IDXA : HC_IDXA + 1], sem_ga, (sem_h, 16))
        gather(GB[:, :], tu2, H[:, HC_IDXB : HC_IDXB + 1], sem_gb)

        # 4 bigram scatters as soon as their offsets + the baseline write land
        g.wait_ge(sv, M_OFFB)
        g.wait_ge(sem_out, 16)
        for col in range(1, 5):
            g.indirect_dma_start(
                out=outp[:, :],
                out_offset=IndirectOffsetOnAxis(ap=OFF[:, col : col + 1], axis=0),
                in_=VAL[:, col : col + 1], in_offset=None,
                bounds_check=BOUNDS, oob_is_err=False,
            ).then_inc(sem_sc, 16)

        # trigram offset assembly + scatter
        g.wait_ge(sv, M_TSID)
        go(g.tensor_add(OFFT[:, :], TSID[:, :], H[:, HC_OFFB2C : HC_OFFB2C + 1]))
        g.wait_ge(sv, M_MT2)
        go(g.tensor_add(OFF[:, 0:1], OFFT[:, :], MT2[:, :]))
        g.wait_ge(sg, 2)
        g.indirect_dma_start(
            out=outp[:, :],
            out_offset=IndirectOffsetOnAxis(ap=OFF[:, 0:1], axis=0),
            in_=VAL[:, 0:1], in_offset=None,
            bounds_check=BOUNDS, oob_is_err=False,
        ).then_inc(sem_sc, 16)
        g.wait_ge(sem_sc, 80)

        # ================= vector =================
        # op 1
        vw((sem_h, 16))
        vo(v.tensor_copy(MS8F[:, :], H[:, HC_MS8 : HC_MS8 + 8]))

        # ops 2..6: match + bconst + dense baseline rows
        vw((sem_ga, 16))
        vo(
            v.tensor_tensor(
                EQ1[:, :], GA[:, A_C1 : A_C1 + 32],
                H[:, HC_H2 : HC_H2 + 1].to_broadcast([128, 32]), OP.is_equal,
            )
        )
        vo(v.tensor_tensor(SCB[:, :], EQ1[:, :], GA[:, A_BW2 : A_BW2 + 32], OP.mult))
        vo(v.tensor_reduce(BW2I[:, :], SCB[:, :], axis=AX.X, op=OP.min))
        vw((sem_gb, 16))
        vo(
            v.tensor_add(
                BCONST[:, :], GB[:, B_BW1 : B_BW1 + 1].bitcast(f32),
                BW2I[:, :].bitcast(f32),
            )
        )
        vw((sem_lu, 16))
        vo(v.tensor_scalar(OUTT[:, :], LU[:, :], BCONST[:, 0:1], None, OP.add))
        assert vcnt[0] == M_OUTT

        # ops 7..10: select the matched candidate's trigram record
        vo(
            v.tensor_tensor(
                SCP[:, :].rearrange("p (f c) -> p f c", c=32),
                EQ1[:, :].unsqueeze(1).to_broadcast([128, 9, 32]),
                GA[:, A_POS : A_POS + 288].rearrange("p (f c) -> p f c", c=32),
                OP.mult,
            )
        )
        vo(
            v.tensor_reduce(
                TBP[:, :], SCP[:, :].rearrange("p (f c) -> p f c", c=32),
                axis=AX.X, op=OP.max,
            )
        )
        vo(
            v.tensor_tensor(
                SCN[:, :].rearrange("p (f c) -> p f c", c=32),
                EQ1[:, :].unsqueeze(1).to_broadcast([128, 8, 32]),
                GA[:, A_NEG : A_NEG + 256].rearrange("p (f c) -> p f c", c=32),
                OP.mult,
            )
        )
        vo(
            v.tensor_reduce(
                TBN[:, :], SCN[:, :].rearrange("p (f c) -> p f c", c=32),
                axis=AX.X, op=OP.min,
            )
        )

        # ops 11..17: bigram corrections
        vo(v.tensor_reduce(EX[:, :], EQ1[:, :], axis=AX.X, op=OP.max))
        vo(
            v.tensor_scalar(
                VAL[:, 1:5], GB[:, B_BL : B_BL + 4].bitcast(f32),
                BW2I[:, 0:1].bitcast(f32), None, OP.add,
            )
        )
        vo(
            v.tensor_tensor(
                OFFBIB[:, :], GB[:, B_BI : B_BI + 4],
                H[:, HC_OFFB : HC_OFFB + 1].to_broadcast([128, 4]), OP.add,
            )
        )
        vo(
            v.tensor_tensor(
                EQALL[:, :].rearrange("p (q k) -> p q k", k=8),
                GB[:, B_BI : B_BI + 4].unsqueeze(2).to_broadcast([128, 4, 8]),
                TBP[:, 0:8].unsqueeze(1).to_broadcast([128, 4, 8]),
                OP.is_equal,
            )
        )
        vo(
            v.tensor_reduce(
                COL[:, :], EQALL[:, :].rearrange("p (q k) -> p q k", k=8),
                axis=AX.X, op=OP.max,
            )
        )
        vo(
            v.tensor_tensor(
                COLE[:, :], COL[:, :], EX[:, 0:1].to_broadcast([128, 4]), OP.mult
            )
        )
        vo(
            v.scalar_tensor_tensor(
                OFF[:, 1:5], COLE[:, :], BIG, OFFBIB[:, :],
                op0=OP.mult, op1=OP.add,
            )
        )
        assert vcnt[0] == M_OFFB

        # ops 18..24: trigram slot selects + masks
        vo(v.tensor_tensor(SC4[:, :], H[:, HC_MS8 : HC_MS8 + 8], TBP[:, 0:8], OP.mult))
        vo(v.tensor_reduce(TSID[:, :], SC4[:, :], axis=AX.X, op=OP.max))
        assert vcnt[0] == M_TSID
        vo(v.tensor_tensor(SC5[:, :], MS8F[:, :], TBN[:, :].bitcast(f32), OP.mult))
        vo(v.tensor_reduce(VAL[:, 0:1], SC5[:, :], axis=AX.X, op=OP.min))
        vo(
            v.tensor_tensor(
                LTT[:, :], H[:, HC_SCOL : HC_SCOL + 1], TBP[:, 8:9], OP.is_lt
            )
        )
        vo(v.tensor_add(LTTEX[:, :], LTT[:, :], EX[:, :]))
        vo(v.tensor_scalar(MT2[:, :], LTTEX[:, :], -BIG, None, OP.mult))
        assert vcnt[0] == M_MT2

    return nc


def _build_tables(pointers, ids, logs):
    """Repack the (query-independent) trie into gather-friendly records."""
    ptr = np.asarray(pointers, dtype=np.int64)
    idsv = np.asarray(ids, dtype=np.int32)
    logsv = np.ascontiguousarray(np.asarray(logs, dtype=np.float32))
    logbits = logsv.view(np.int32)

    u = np.arange(U, dtype=np.int64)
    fc = u + ptr[:U]
    num = (ptr[1 : U + 1] - ptr[:U] + 1).astype(np.int64)
    ar32 = np.arange(32, dtype=np.int64)

    cn = np.clip(fc[:, None] + ar32[None, :], U, U + B2 - 1)   # [U,32] bigram nodes
    c_ids = idsv[cn - U].astype(np.int32)
    invalid1 = ar32[None, :] >= num[:, None]

    tu1 = np.zeros((U, W1), dtype=np.int32)
    tu1[:, A_C1 : A_C1 + 32] = np.where(invalid1, -1, c_ids)
    tu1[:, A_BW2 : A_BW2 + 32] = logbits[NNODES + cn]

    fc3 = cn + ptr[cn]                                          # [U,32]
    num3 = (ptr[cn + 1] - ptr[cn] + 1).astype(np.int32)
    cn3 = np.clip(
        fc3[:, :, None] + np.arange(8)[None, None, :], U + B2, NNODES - 1
    )                                                           # [U,32,8]
    tid = idsv[cn3 - U].astype(np.int32)
    tid = np.where(np.arange(8)[None, None, :] >= num3[:, :, None], -2, tid)
    tlb = logbits[cn3]

    pos = np.concatenate(
        [tid.transpose(0, 2, 1), num3[:, None, :]], axis=1
    )                                                           # [U,9,32]
    tu1[:, A_POS : A_POS + 288] = pos.reshape(U, 288)
    tu1[:, A_NEG : A_NEG + 256] = tlb.transpose(0, 2, 1).reshape(U, 256)

    bi_logs = logbits[cn]
    bi_ids = np.where(invalid1, BADID, c_ids)
    tu2 = np.zeros((U, W2), dtype=np.int32)
    for s in range(8):
        tu2[:, 10 * s + B_BI : 10 * s + B_BI + 4] = bi_ids[:, 4 * s : 4 * s + 4]
        tu2[:, 10 * s + B_BL : 10 * s + B_BL + 4] = bi_logs[:, 4 * s : 4 * s + 4]
        tu2[:, 10 * s + B_NUM2] = num.astype(np.int32)
        tu2[:, 10 * s + B_BW1] = logbits[NNODES + u]

    lurep = np.ascontiguousarray(
        np.tile(logsv[:V].reshape(8, 1024), (16, 1)).astype(np.float32)
    )
    return (
        np.ascontiguousarray(tu1.reshape(U * W1, 1)),
        np.ascontiguousarray(tu2.reshape(U * W2, 1)),
        lurep,
    )


def _prep_in_maps(hist, idx, pointers, ids, logs):
    hist = np.asarray(hist)
    idxi = int(np.asarray(idx))
    hh = hist[:idxi][-(N - 1):]
    assert hh.shape == (2, BATCH), hh.shape
    tu1, tu2, lurep = _build_tables(pointers, ids, logs)

    p = np.arange(128)
    b = p >> 3
    s = p & 7
    hc_base = np.zeros((128, HC), dtype=np.int64)
    hc_base[:, HC_MS8 : HC_MS8 + 8] = (np.arange(8)[None, :] == s[:, None])
    hc_base[:, HC_OFFB] = b << 13
    hc_base[:, HC_OFFB2C] = (b << 13) + 2 * BIG
    hc_base[:, HC_SCOL] = s

    in_maps = []
    for c in range(NCORES):
        sl = hh[:, c * BPC : (c + 1) * BPC].astype(np.int64)
        hc = hc_base.copy()
        h1 = sl[0][b]
        h2 = sl[1][b]
        hc[:, HC_IDXA] = h1 * W1
        hc[:, HC_IDXB] = h2 * W2 + 10 * s
        hc[:, HC_H2] = h2
        in_maps.append(
            {
                "hconst": np.ascontiguousarray(hc.astype(np.int32)),
                "lurep": lurep,
                "tu1": tu1,
                "tu2": tu2,
            }
        )
    return in_maps


def _assemble(results):
    return np.concatenate(
        [results[c]["out"].reshape(BPC, V) for c in range(NCORES)], axis=0
    )


def kernel(hist, idx, pointers, ids, logs):
    nc = build_kernel()
    in_maps = _prep_in_maps(hist, idx, pointers, ids, logs)
    res = run_bass_kernel_spmd(nc, in_maps, list(range(NCORES)))
    return _assemble(res.results)


def kernel_timed(hist, idx, pointers, ids, logs, trace=True):
    """Like kernel() but returns (output, BassKernelResults) with trace."""
    nc = build_kernel()
    in_maps = _prep_in_maps(hist, idx, pointers, ids, logs)
    res = run_bass_kernel_spmd(nc, in_maps, list(range(NCORES)), trace=trace)
    return _assemble(res.results), res


# revision 7
# speedup vs baseline: 1.1005x; 1.1005x over previous
"""Trainium2 Bass kernel for nn_LookupLanguageModel (trigram backoff LM lookup).

v4: two gather rounds, no chained trigram round. HW indirect DMA consumes one
offset per partition, so all merging is done in the HOST TABLE LAYOUT
(query-independent repack of the read-only trie, per the replicate-the-tables
sharding hint):

  TU1[u] (608 i32), keyed by h1:
    [0:32)    ids of u's 32 children (invalid slots baked to -1: never match)
    [32:64)   children's backoff logs (f32 bits; for bw2 select)
    [64:352)  field-major POS block [9 fields x 32 candidates]: for candidate
              c, the 8 trigram-child ids of bigram node (h1->c) (f=0..7) and
              its num_children (f=8)
    [352:608) field-major NEG block [8 x 32]: the 8 trigram-child log bits
  TU2[u] (80 i32), keyed by (h2, slot): 8 blocks of [4 bigram-candidate ids
    (invalid baked to 2^17: out of scatter bounds), their 4 log bits, num2,
    bw1 bits]

After G_A=TU1[h1] and G_B=TU2[h2] land, the matched candidate cm (ids==h2) is
selected by masked mult + reduce along the candidate axis: max for the POS
fields (ids >= 0), min for NEG/bw2 (log bit patterns of negative floats are
negative ints -- exact, no float accumulation). That yields the entire
trigram record without another DMA round. Corrections go out as five
128-offset scatters (4 bigram + 1 trigram per partition), invalid/collided
slots pushed out of range and dropped via bounds_check. The dense baseline
write (logs[v]+bconst, from a host-replicated logs[0:V]) issues as vector op 6
so its completion no longer gates the scatters.

Layout: 128 partitions = 16 rows x 8 slots; partition p: row b=p>>3, slot
s=p&7 (output chunk [1024s,1024s+1024), trigram candidate s, bigram
candidates 4s..4s+3). Core c handles batch rows 16c..16c+15.
"""

import numpy as np

import concourse.bass as bass
import concourse.mybir as mybir
from concourse.bass import IndirectOffsetOnAxis
from concourse.bass_utils import run_bass_kernel_spmd

# ---- problem constants (must match the reference trie shapes) ----
V = 8192
N = 3
U = V + 1                   # 8193 unigram nodes
C2, C3 = 32, 8
B2 = U * C2                 # 262176 bigram nodes
B3 = B2 * C3                # 2097408 trigram nodes
XP = U + B2 + 1             # pointers length 270370
KI = B2 + B3                # ids length 2359584
NNODES = U + B2 + B3        # 2367777 (start of backoff weights inside logs)
LL = 2 * XP + (B3 - 1)      # logs length 2638147
BATCH = 128
NCORES = 8
BPC = BATCH // NCORES       # 16 rows per core

W1 = 608                    # TU1 record width
W2 = 80                     # TU2 record width

BIG = 1 << 18               # offset mask-out constant (> BPC*V - 1)
BADID = 1 << 17             # baked id for invalid bigram candidates
BOUNDS = BPC * V - 1        # max valid flat output element index per core

i32 = mybir.dt.int32
f32 = mybir.dt.float32

AX = mybir.AxisListType
OP = mybir.AluOpType

# hconst column map (int32 [128, HC])
HC_IDXA = 0      # h1 * W1
HC_IDXB = 1      # h2 * W2 + 10*s
HC_H2 = 2
HC_MS8 = 3       # 8 cols: (k==s)
HC_OFFB = 11     # b<<13
HC_OFFB2C = 12   # (b<<13) + 2*BIG
HC_SCOL = 13
HC = 64    # padded: fatter rows keep the descriptor stream well-batched

# TU1 record layout
A_C1 = 0
A_BW2 = 32
A_POS = 64       # 9 fields x 32 candidates (tri ids f=0..7, num3 f=8)
A_NEG = 352      # 8 fields x 32 candidates (tri log bits)

# TU2 slot-block layout (10 per slot)
B_BI = 0
B_BL = 4
B_NUM2 = 8
B_BW1 = 9


def build_kernel() -> bass.Bass:
    nc = bass.Bass()

    hconst = nc.declare_dram_parameter("hconst", [128, HC], i32, isOutput=False)
    lurep = nc.declare_dram_parameter("lurep", [128, 1024], f32, isOutput=False)
    tu1 = nc.declare_dram_parameter("tu1", [U * W1, 1], i32, isOutput=False)
    tu2 = nc.declare_dram_parameter("tu2", [U * W2, 1], i32, isOutput=False)
    outp = nc.declare_dram_parameter("out", [BPC * V, 1], f32, isOutput=True)

    from contextlib import ExitStack

    with ExitStack() as ctx:
        _n = [0]

        def sb(shape, dt):
            _n[0] += 1
            return ctx.enter_context(nc.sbuf_tensor(f"t{_n[0]}", shape, dt))

        H = sb([128, HC], i32)
        LU = sb([128, 1024], f32)
        OUTT = sb([128, 1024], f32)
        GA = sb([128, W1], i32)
        GB = sb([128, 10], i32)
        OFF = sb([128, 5], i32)
        VAL = sb([128, 5], f32)

        MS8F = sb([128, 8], f32)
        EQ1 = sb([128, 32], i32)
        SCB = sb([128, 32], i32)
        SCP = sb([128, 288], i32)
        SCN = sb([128, 256], i32)
        TBP = sb([128, 9], i32)
        TBN = sb([128, 8], i32)
        SC4 = sb([128, 8], i32)
        SC5 = sb([128, 8], f32)
        EX = sb([128, 1], i32)
        BW2I = sb([128, 1], i32)
        BCONST = sb([128, 1], f32)
        TSID = sb([128, 1], i32)
        OFFT = sb([128, 1], i32)
        LTT = sb([128, 1], i32)
        LTTEX = sb([128, 1], i32)
        MT2 = sb([128, 1], i32)
        OFFBIB = sb([128, 4], i32)
        EQALL = sb([128, 32], i32)
        COL = sb([128, 4], i32)
        COLE = sb([128, 4], i32)

        sem = lambda name: ctx.enter_context(nc.semaphore(name))
        sv = sem("sv")
        sg = sem("sg")
        sem_h = sem("sem_h")
        sem_lu = sem("sem_lu")
        sem_ga = sem("sem_ga")
        sem_gb = sem("sem_gb")
        sem_out = sem("sem_out")
        sem_sc = sem("sem_sc")

        ctx.enter_context(nc.Block())

        g = nc.gpsimd
        v = nc.vector
        sy = nc.sync

        vcnt = [0]

        def vo(inst):
            if vcnt[0] > 0:
                inst.wait_op(sv, vcnt[0], "sem-ge")
            inst.then_inc(sv, 1)
            vcnt[0] += 1
            return inst

        def vw(*waits):
            for s_, val_ in waits:
                v.wait_ge(s_, val_)

        gcnt = [0]

        def go(inst):
            if gcnt[0] > 0:
                inst.wait_op(sg, gcnt[0], "sem-ge")
            inst.then_inc(sg, 1)
            gcnt[0] += 1
            return inst

        M_OUTT = 6
        M_OFFB = 17
        M_TSID = 19
        M_MT2 = 24

        # ============ sync/scalar: input DMAs + baseline write ============
        # lurep goes out on the Scalar engine's queues so its 512KB stream
        # cannot delay hconst's descriptors (observed straggling on HW).
        sy.dma_start(out=H[:, :], in_=hconst[:, :]).then_inc(sem_h, 16)
        nc.scalar.dma_start(out=LU[:, :], in_=lurep[:, :]).then_inc(sem_lu, 16)

        sy.wait_ge(sv, M_OUTT)
        sy.dma_start(
            out=outp[:, :].rearrange("(p f) o -> p (f o)", p=128),
            in_=OUTT[:, :],
        ).then_inc(sem_out, 16)

        # ================= gpsimd: 2 gathers + tail + 5 scatters =============
        def gather(dst, src, idx_ap, semh, *waits):
            for s_, val_ in waits:
                g.wait_ge(s_, val_)
            inst = g.indirect_dma_start(
                out=dst, out_offset=None,
                in_=src[:, :], in_offset=IndirectOffsetOnAxis(ap=idx_ap, axis=0),
            )
            inst.then_inc(semh, 16)
            return inst

        gather(GA[:, :], tu1, H[:, HC_IDXA : HC_IDXA + 1], sem_ga, (sem_h, 16))
        gather(GB[:, :], tu2, H[:, HC_IDXB : HC_IDXB + 1], sem_gb)

        # 4 bigram scatters as soon as their offsets + the baseline write land
        g.wait_ge(sv, M_OFFB)
        g.wait_ge(sem_out, 16)
        for col in range(1, 5):
            g.indirect_dma_start(
                out=outp[:, :],
                out_offset=IndirectOffsetOnAxis(ap=OFF[:, col : col + 1], axis=0),
                in_=VAL[:, col : col + 1], in_offset=None,
                bounds_check=BOUNDS, oob_is_err=False,
            ).then_inc(sem_sc, 16)

        # trigram offset assembly + scatter
        g.wait_ge(sv, M_TSID)
        go(g.tensor_add(OFFT[:, :], TSID[:, :], H[:, HC_OFFB2C : HC_OFFB2C + 1]))
        g.wait_ge(sv, M_MT2)
        go(g.tensor_add(OFF[:, 0:1], OFFT[:, :], MT2[:, :]))
        g.wait_ge(sg, 2)
        g.indirect_dma_start(
            out=outp[:, :],
            out_offset=IndirectOffsetOnAxis(ap=OFF[:, 0:1], axis=0),
            in_=VAL[:, 0:1], in_offset=None,
            bounds_check=BOUNDS, oob_is_err=False,
        ).then_inc(sem_sc, 16)
        # no explicit wait on sem_sc: the end-of-kernel DRAIN blocks until the
        # scatter queues are empty, saving the semaphore round-trip.

        # ================= vector =================
        # op 1
        vw((sem_h, 16))
        vo(v.tensor_copy(MS8F[:, :], H[:, HC_MS8 : HC_MS8 + 8]))

        # ops 2..6: match + bconst + dense baseline rows
        vw((sem_ga, 16))
        vo(
            v.tensor_tensor(
                EQ1[:, :], GA[:, A_C1 : A_C1 + 32],
                H[:, HC_H2 : HC_H2 + 1].to_broadcast([128, 32]), OP.is_equal,
            )
        )
        vo(v.tensor_tensor(SCB[:, :], EQ1[:, :], GA[:, A_BW2 : A_BW2 + 32], OP.mult))
        vo(v.tensor_reduce(BW2I[:, :], SCB[:, :], axis=AX.X, op=OP.min))
        vw((sem_gb, 16))
        vo(
            v.tensor_add(
                BCONST[:, :], GB[:, B_BW1 : B_BW1 + 1].bitcast(f32),
                BW2I[:, :].bitcast(f32),
            )
        )
        vw((sem_lu, 16))
        vo(v.tensor_scalar(OUTT[:, :], LU[:, :], BCONST[:, 0:1], None, OP.add))
        assert vcnt[0] == M_OUTT

        # ops 7..10: select the matched candidate's trigram record
        vo(
            v.tensor_tensor(
                SCP[:, :].rearrange("p (f c) -> p f c", c=32),
                EQ1[:, :].unsqueeze(1).to_broadcast([128, 9, 32]),
                GA[:, A_POS : A_POS + 288].rearrange("p (f c) -> p f c", c=32),
                OP.mult,
            )
        )
        vo(
            v.tensor_reduce(
                TBP[:, :], SCP[:, :].rearrange("p (f c) -> p f c", c=32),
                axis=AX.X, op=OP.max,
            )
        )
        vo(
            v.tensor_tensor(
                SCN[:, :].rearrange("p (f c) -> p f c", c=32),
                EQ1[:, :].unsqueeze(1).to_broadcast([128, 8, 32]),
                GA[:, A_NEG : A_NEG + 256].rearrange("p (f c) -> p f c", c=32),
                OP.mult,
            )
        )
        vo(
            v.tensor_reduce(
                TBN[:, :], SCN[:, :].rearrange("p (f c) -> p f c", c=32),
                axis=AX.X, op=OP.min,
            )
        )

        # ops 11..17: bigram corrections
        vo(v.tensor_reduce(EX[:, :], EQ1[:, :], axis=AX.X, op=OP.max))
        vo(
            v.tensor_scalar(
                VAL[:, 1:5], GB[:, B_BL : B_BL + 4].bitcast(f32),
                BW2I[:, 0:1].bitcast(f32), None, OP.add,
            )
        )
        vo(
            v.tensor_tensor(
                OFFBIB[:, :], GB[:, B_BI : B_BI + 4],
                H[:, HC_OFFB : HC_OFFB + 1].to_broadcast([128, 4]), OP.add,
            )
        )
        vo(
            v.tensor_tensor(
                EQALL[:, :].rearrange("p (q k) -> p q k", k=8),
                GB[:, B_BI : B_BI + 4].unsqueeze(2).to_broadcast([128, 4, 8]),
                TBP[:, 0:8].unsqueeze(1).to_broadcast([128, 4, 8]),
                OP.is_equal,
            )
        )
        vo(
            v.tensor_reduce(
                COL[:, :], EQALL[:, :].rearrange("p (q k) -> p q k", k=8),
                axis=AX.X, op=OP.max,
            )
        )
        vo(
            v.tensor_tensor(
                COLE[:, :], COL[:, :], EX[:, 0:1].to_broadcast([128, 4]), OP.mult
            )
        )
        vo(
            v.scalar_tensor_tensor(
                OFF[:, 1:5], COLE[:, :], BIG, OFFBIB[:, :],
                op0=OP.mult, op1=OP.add,
            )
        )
        assert vcnt[0] == M_OFFB

        # ops 18..24: trigram slot selects + masks
        vo(v.tensor_tensor(SC4[:, :], H[:, HC_MS8 : HC_MS8 + 8], TBP[:, 0:8], OP.mult))
        vo(v.tensor_reduce(TSID[:, :], SC4[:, :], axis=AX.X, op=OP.max))
        assert vcnt[0] == M_TSID
        vo(v.tensor_tensor(SC5[:, :], MS8F[:, :], TBN[:, :].bitcast(f32), OP.mult))
        vo(v.tensor_reduce(VAL[:, 0:1], SC5[:, :], axis=AX.X, op=OP.min))
        vo(
            v.tensor_tensor(
                LTT[:, :], H[:, HC_SCOL : HC_SCOL + 1], TBP[:, 8:9], OP.is_lt
            )
        )
        vo(v.tensor_add(LTTEX[:, :], LTT[:, :], EX[:, :]))
        vo(v.tensor_scalar(MT2[:, :], LTTEX[:, :], -BIG, None, OP.mult))
        assert vcnt[0] == M_MT2

    return nc


def _build_tables(pointers, ids, logs):
    """Repack the (query-independent) trie into gather-friendly records."""
    ptr = np.asarray(pointers, dtype=np.int64)
    idsv = np.asarray(ids, dtype=np.int32)
    logsv = np.ascontiguousarray(np.asarray(logs, dtype=np.float32))
    logbits = logsv.view(np.int32)

    u = np.arange(U, dtype=np.int64)
    fc = u + ptr[:U]
    num = (ptr[1 : U + 1] - ptr[:U] + 1).astype(np.int64)
    ar32 = np.arange(32, dtype=np.int64)

    cn = np.clip(fc[:, None] + ar32[None, :], U, U + B2 - 1)   # [U,32] bigram nodes
    c_ids = idsv[cn - U].astype(np.int32)
    invalid1 = ar32[None, :] >= num[:, None]

    tu1 = np.zeros((U, W1), dtype=np.int32)
    tu1[:, A_C1 : A_C1 + 32] = np.where(invalid1, -1, c_ids)
    tu1[:, A_BW2 : A_BW2 + 32] = logbits[NNODES + cn]

    fc3 = cn + ptr[cn]                                          # [U,32]
    num3 = (ptr[cn + 1] - ptr[cn] + 1).astype(np.int32)
    cn3 = np.clip(
        fc3[:, :, None] + np.arange(8)[None, None, :], U + B2, NNODES - 1
    )                                                           # [U,32,8]
    tid = idsv[cn3 - U].astype(np.int32)
    tid = np.where(np.arange(8)[None, None, :] >= num3[:, :, None], -2, tid)
    tlb = logbits[cn3]

    pos = np.concatenate(
        [tid.transpose(0, 2, 1), num3[:, None, :]], axis=1
    )                                                           # [U,9,32]
    tu1[:, A_POS : A_POS + 288] = pos.reshape(U, 288)
    tu1[:, A_NEG : A_NEG + 256] = tlb.transpose(0, 2, 1).reshape(U, 256)

    bi_logs = logbits[cn]
    bi_ids = np.where(invalid1, BADID, c_ids)
    tu2 = np.zeros((U, W2), dtype=np.int32)
    for s in range(8):
        tu2[:, 10 * s + B_BI : 10 * s + B_BI + 4] = bi_ids[:, 4 * s : 4 * s + 4]
        tu2[:, 10 * s + B_BL : 10 * s + B_BL + 4] = bi_logs[:, 4 * s : 4 * s + 4]
        tu2[:, 10 * s + B_NUM2] = num.astype(np.int32)
        tu2[:, 10 * s + B_BW1] = logbits[NNODES + u]

    lurep = np.ascontiguousarray(
        np.tile(logsv[:V].reshape(8, 1024), (16, 1)).astype(np.float32)
    )
    return (
        np.ascontiguousarray(tu1.reshape(U * W1, 1)),
        np.ascontiguousarray(tu2.reshape(U * W2, 1)),
        lurep,
    )


def _prep_in_maps(hist, idx, pointers, ids, logs):
    hist = np.asarray(hist)
    idxi = int(np.asarray(idx))
    hh = hist[:idxi][-(N - 1):]
    assert hh.shape == (2, BATCH), hh.shape
    tu1, tu2, lurep = _build_tables(pointers, ids, logs)

    p = np.arange(128)
    b = p >> 3
    s = p & 7
    hc_base = np.zeros((128, HC), dtype=np.int64)
    hc_base[:, HC_MS8 : HC_MS8 + 8] = (np.arange(8)[None, :] == s[:, None])
    hc_base[:, HC_OFFB] = b << 13
    hc_base[:, HC_OFFB2C] = (b << 13) + 2 * BIG
    hc_base[:, HC_SCOL] = s

    in_maps = []
    for c in range(NCORES):
        sl = hh[:, c * BPC : (c + 1) * BPC].astype(np.int64)
        hc = hc_base.copy()
        h1 = sl[0][b]
        h2 = sl[1][b]
        hc[:, HC_IDXA] = h1 * W1
        hc[:, HC_IDXB] = h2 * W2 + 10 * s
        hc[:, HC_H2] = h2
        in_maps.append(
            {
                "hconst": np.ascontiguousarray(hc.astype(np.int32)),
                "lurep": lurep,
                "tu1": tu1,
                "tu2": tu2,
            }
        )
    return in_maps


def _assemble(results):
    return np.concatenate(
        [results[c]["out"].reshape(BPC, V) for c in range(NCORES)], axis=0
    )


def kernel(hist, idx, pointers, ids, logs):
    nc = build_kernel()
    in_maps = _prep_in_maps(hist, idx, pointers, ids, logs)
    res = run_bass_kernel_spmd(nc, in_maps, list(range(NCORES)))
    return _assemble(res.results)


def kernel_timed(hist, idx, pointers, ids, logs, trace=True):
    """Like kernel() but returns (output, BassKernelResults) with trace."""
    nc = build_kernel()
    in_maps = _prep_in_maps(hist, idx, pointers, ids, logs)
    res = run_bass_kernel_spmd(nc, in_maps, list(range(NCORES)), trace=trace)
    return _assemble(res.results), res
